# revision 16
# baseline (speedup 1.0000x reference)
"""Trainium2 Bass kernel for the Combine-Attention layer.

Math (per batch b, head h; N=512 nodes, D=6 dot dim):
    qkv_s = s @ Ws + bs ; qkv_t = t @ Wt + bt          (projections)
    As_hat[l,m] = (Qs.Kt)[l,m]*scale + E[l,m]
    At_hat[l,m] = (Qt.Ks)[l,m]*scale + E[m,l]
    As~ = As_hat * Dst ;  At~[l,m] = At_hat[l,m]*Dst[m,l]
    p_s = sigmoid(As~ - At~) ; p_t = 1 - p_s
    W1 = p_s * sigmoid(G) ; W2[l,m] = p_t[l,m] * sigmoid(G[m,l])
    Y = W1 @ Vt + W2 @ Vs
Outputs: (Y reshaped [n, H*D], As_hat [n,n,H]).

Device strategy: data-parallel over batch (1 batch per NeuronCore, 8 cores).
All score-space work is done in the transposed orientation [m, l]:
    U[l,m]    := At~[m,l] = ((Ks.Qt)[l,m]*scale + E[l,m]) * Dst[l,m]
    diffT[m,l] = A^T[m,l] - U[m,l]        (A := As~; A^T via PE transposes)
    W1T[m,l]  = sigmoid(diffT)*gs^T[m,l] ; W2T[m,l] = sigmoid(-diffT)*gs[m,l]
    Y^T[k,l]  = sum_m Vt[m,k]*W1T[m,l] + Vs[m,k]*W2T[m,l]
so every DRAM access is layout-native (host pre-permutes E,G to [H,N,N] and
the QKV weights into head-major column order with bias folded in as an
extra contraction row).
"""

import os
import numpy as np

B, N, NW, H, D = 8, 512, 48, 8, 6
P = 128
NT = N // P          # 4 strips of 128 rows
SCALE = D ** -0.5
NCORES = 8

_CACHE = {}
LAST_RESULTS = None


# ----------------------------------------------------------------- host prep
def _prep_weights(Ws, bs, Wt, bt):
    """Fold projections into per-head score matrices + V weights.

    Original column c = (x*6 + d)*8 + h for x in {Q=0,K=1,V=2}.
    M1_h = scale * Wq_s_aug @ Wk_t_aug^T  (49x49):  S1  = s' M1 t'^T
    M2_h = scale * Wk_s_aug @ Wq_t_aug^T  (49x49):  S2T = s' M2 t'^T
    Packed transposed per head-pair into [49, 4, 128] (head 2p at free cols
    0:49, head 2p+1 at 64:113) so psum rows land at legal matmul bases.
    WV_aug [49, 48] per side, V column j = h*6 + k.
    """
    def aug(W, b, cols):
        return np.vstack([W[:, cols], b[cols][None, :]]).astype(np.float64)

    Ws = np.asarray(Ws, np.float64)
    bs = np.asarray(bs, np.float64)
    Wt = np.asarray(Wt, np.float64)
    bt = np.asarray(bt, np.float64)
    M1T = np.zeros((NW + 1, NT, P), np.float32)
    M2T = np.zeros((NW + 1, NT, P), np.float32)
    for h in range(H):
        qcols = [(0 * D + d) * H + h for d in range(D)]
        kcols = [(1 * D + d) * H + h for d in range(D)]
        wq_s = aug(Ws, bs, qcols)
        wk_s = aug(Ws, bs, kcols)
        wq_t = aug(Wt, bt, qcols)
        wk_t = aug(Wt, bt, kcols)
        m1 = (SCALE * wq_s @ wk_t.T).T.astype(np.float32)
        m2 = (SCALE * wk_s @ wq_t.T).T.astype(np.float32)
        p, r = divmod(h, 2)
        M1T[:, p, r * 64 : r * 64 + NW + 1] = m1
        M2T[:, p, r * 64 : r * 64 + NW + 1] = m2
    v_cols = [(2 * D + k) * H + h for h in range(H) for k in range(D)]
    WVs = np.ascontiguousarray(aug(Ws, bs, v_cols), dtype=np.float32)
    WVt = np.ascontiguousarray(aug(Wt, bt, v_cols), dtype=np.float32)
    return M1T, M2T, WVs, WVt


# ------------------------------------------------------------- device kernel
def _emit(ctx, tc, nc, io):
    import concourse.bass as bass
    import concourse.mybir as mybir
    from concourse.masks import make_identity

    f32 = mybir.dt.float32
    AF = mybir.ActivationFunctionType
    OP = mybir.AluOpType

    E_t, G_t, Dst_d, s_d, t_d, M1T_d, M2T_d, WVs_d, WVt_d, ones_d, Y_d, AsH_d = io

    singles = ctx.enter_context(tc.tile_pool(name="singles", bufs=1))

    ident = singles.tile([P, P], f32)
    make_identity(nc, ident)

    # Dst strips resident for the whole batch: [128, strip, 512]
    Dst_sb = singles.tile([P, NT, N], f32)
    nc.sync.dma_start(out=Dst_sb, in_=Dst_d.ap().rearrange("(a p) m -> p a m", p=P))

    # --- projections -------------------------------------------------------
    xTs = {}    # side -> [49, 512] sbuf (transposed inputs, ones row appended)
    Vnat = {}   # side -> [128, strip, 48] sbuf (cols: h*6 + k)
    R = {}      # (1|2, pair) -> [128, 512] sbuf rows r*64..r*64+48 = head 2p+r
    with tc.tile_pool(name="proj_ps", bufs=2, space="PSUM") as proj_ps:
        for side, src, Wv_d in (("s", s_d, WVs_d), ("t", t_d, WVt_d)):
            wv = singles.tile([NW + 1, NW], f32, tag=f"wv_{side}")
            nc.sync.dma_start(out=wv, in_=Wv_d.ap())

            x_nat = singles.tile([P, NT, NW], f32, tag=f"xnat_{side}")
            nc.sync.dma_start(
                out=x_nat, in_=src.ap().rearrange("(a p) c -> p a c", p=P)
            )
            # rows 0:49 = [x^T; ones]; for side s, duplicated at partition 64
            # so score matmuls match the base partition of odd-head R slabs
            nrows = 64 + NW + 1 if side == "s" else NW + 1
            xT = singles.tile([nrows, N], f32, tag=f"xT_{side}")
            # engine memset at partition 48 is illegal (32-aligned bases
            # only) -- DMA the ones row in instead
            nc.sync.dma_start(out=xT[NW : NW + 1, :], in_=ones_d.ap())
            for a in range(NT):
                ps = proj_ps.tile([NW, P], f32, tag="tr")
                nc.tensor.transpose(ps, x_nat[:, a, :], ident)
                nc.vector.tensor_copy(xT[0:NW, a * P : (a + 1) * P], ps)
            if side == "s":
                nc.sync.dma_start(
                    out=xT[64 : 64 + NW + 1, :], in_=xT[0 : NW + 1, :]
                )
            xTs[side] = xT

            Vnat[side] = singles.tile(
                [P, NT, NW], f32, tag=f"vnat_{side}", name=f"vnat_{side}"
            )
            for a in range(NT):
                vps = proj_ps.tile([P, NW], f32, tag="v")
                nc.tensor.matmul(
                    vps, lhsT=xT[0 : NW + 1, a * P : (a + 1) * P], rhs=wv,
                    start=True, stop=True,
                )
                nc.vector.tensor_copy(Vnat[side][:, a, :], vps)

        # R1/R2: per head-pair p, rows r*64..r*64+48 hold M @ t'^T for head 2p+r
        for idx, M_d in ((1, M1T_d), (2, M2T_d)):
            m_sb = singles.tile(
                [NW + 1, NT, P], f32, tag=f"m{idx}", name=f"m{idx}_sb"
            )
            nc.sync.dma_start(out=m_sb, in_=M_d.ap())
            for p in range(NT):
                rps = proj_ps.tile([P, N], f32, tag="r")
                nc.tensor.matmul(
                    rps, lhsT=m_sb[:, p, :], rhs=xTs["t"], start=True, stop=True
                )
                r_sb = singles.tile(
                    [P, N], f32, tag=f"r{idx}_{p}", name=f"r{idx}_{p}"
                )
                nc.vector.tensor_copy(r_sb, rps)
                R[(idx, p)] = r_sb

    Ystage = singles.tile([P, NT, H * D], f32)

    score_ps = ctx.enter_context(tc.tile_pool(name="score_ps", bufs=2, space="PSUM"))
    trans_ps = ctx.enter_context(tc.tile_pool(name="trans_ps", bufs=1, space="PSUM"))
    y_ps = ctx.enter_context(tc.tile_pool(name="y_ps", bufs=1, space="PSUM"))
    big = ctx.enter_context(tc.tile_pool(name="big", bufs=5))
    med = ctx.enter_context(tc.tile_pool(name="med", bufs=2))
    io_pool = ctx.enter_context(tc.tile_pool(name="io", bufs=3))

    for h in range(H):
        pair, r = divmod(h, 2)
        R1_h = R[(1, pair)][r * 64 : r * 64 + NW + 1, :]
        R2_h = R[(2, pair)][r * 64 : r * 64 + NW + 1, :]
        sT = xTs["s"][r * 64 : r * 64 + NW + 1, :]

        A_s, U_s, gs_s = [], [], []
        for i in range(NT):
            cols = slice(i * P, (i + 1) * P)
            E_i = io_pool.tile([P, N], f32, tag="E")
            nc.sync.dma_start(out=E_i, in_=E_t.ap()[h, i * P : (i + 1) * P, :])
            G_i = io_pool.tile([P, N], f32, tag="G")
            nc.sync.dma_start(out=G_i, in_=G_t.ap()[h, i * P : (i + 1) * P, :])

            ps1 = score_ps.tile([P, N], f32, tag="ps1")
            nc.tensor.matmul(ps1, lhsT=sT[:, cols], rhs=R1_h, start=True, stop=True)
            As_i = io_pool.tile([P, N], f32, tag="As")
            # As_hat = S1 + E  (fused psum-read + E add)
            nc.vector.scalar_tensor_tensor(
                out=As_i, in0=ps1, scalar=1.0, in1=E_i, op0=OP.mult, op1=OP.add
            )
            nc.sync.dma_start(out=AsH_d.ap()[h, i * P : (i + 1) * P, :], in_=As_i)

            A_i = big.tile([P, N], f32, tag="A")
            nc.vector.tensor_mul(A_i, As_i, Dst_sb[:, i, :])
            A_s.append(A_i)

            ps2 = score_ps.tile([P, N], f32, tag="ps2")
            nc.tensor.matmul(ps2, lhsT=sT[:, cols], rhs=R2_h, start=True, stop=False)
            nc.tensor.matmul(ps2, lhsT=ident, rhs=E_i, start=False, stop=True)
            U_i = big.tile([P, N], f32, tag="U")
            nc.vector.tensor_mul(U_i, ps2, Dst_sb[:, i, :])
            U_s.append(U_i)

            gs_i = big.tile([P, N], f32, tag="gs")
            nc.scalar.activation(gs_i, G_i, AF.Sigmoid)
            gs_s.append(gs_i)

        W1T_s, W2T_s = [], []
        for j in range(NT):
            jcols = slice(j * P, (j + 1) * P)
            atp = trans_ps.tile([P, N], f32, tag="at")
            gtp = trans_ps.tile([P, N], f32, tag="gt")
            for i in range(NT):
                icols = slice(i * P, (i + 1) * P)
                nc.tensor.transpose(atp[:, icols], A_s[i][:, jcols], ident)
                nc.tensor.transpose(gtp[:, icols], gs_s[i][:, jcols], ident)
            diff = med.tile([P, N], f32, tag="diff")
            nc.vector.tensor_sub(diff, atp, U_s[j])
            p_j = med.tile([P, N], f32, tag="p")
            nc.scalar.activation(p_j, diff, AF.Sigmoid)
            q_j = med.tile([P, N], f32, tag="q")
            nc.scalar.activation(q_j, diff, AF.Sigmoid, scale=-1.0)
            W1T_j = big.tile([P, N], f32, tag="W1")
            nc.vector.tensor_mul(W1T_j, p_j, gtp)
            W1T_s.append(W1T_j)
            W2T_j = big.tile([P, N], f32, tag="W2")
            nc.vector.tensor_mul(W2T_j, q_j, gs_s[j])
            W2T_s.append(W2T_j)

        yt = y_ps.tile([D, N], f32, tag="yt")
        for j in range(NT):
            nc.tensor.matmul(
                yt, lhsT=Vnat["t"][:, j, h * D : (h + 1) * D], rhs=W1T_s[j],
                start=(j == 0), stop=False,
            )
            nc.tensor.matmul(
                yt, lhsT=Vnat["s"][:, j, h * D : (h + 1) * D], rhs=W2T_s[j],
                start=False, stop=(j == NT - 1),
            )
        yt_sb = med.tile([D, N], f32, tag="ytsb")
        nc.vector.tensor_copy(yt_sb, yt)
        yt2 = y_ps.tile([P, NT, D], f32, tag="yt2")
        for i in range(NT):
            nc.tensor.transpose(
                yt2[:, i, :], yt_sb[:, i * P : (i + 1) * P], ident[:D, :D]
            )
        # Ystage columns k*8+h  <-  yt2 columns k
        nc.vector.tensor_copy(Ystage[:, :, h :: H], yt2)

    nc.sync.dma_start(
        out=Y_d.ap().rearrange("(a p) c -> p a c", p=P), in_=Ystage
    )


def _build():
    from contextlib import ExitStack

    import concourse.bacc as bacc
    import concourse.mybir as mybir
    import concourse.tile as tile

    f32 = mybir.dt.float32
    nc = bacc.Bacc("TRN2", target_bir_lowering=False, debug=False)
    io = (
        nc.dram_tensor("E_t", [H, N, N], f32, kind="ExternalInput"),
        nc.dram_tensor("G_t", [H, N, N], f32, kind="ExternalInput"),
        nc.dram_tensor("Dst", [N, N], f32, kind="ExternalInput"),
        nc.dram_tensor("s", [N, NW], f32, kind="ExternalInput"),
        nc.dram_tensor("t", [N, NW], f32, kind="ExternalInput"),
        nc.dram_tensor("M1T", [NW + 1, NT, P], f32, kind="ExternalInput"),
        nc.dram_tensor("M2T", [NW + 1, NT, P], f32, kind="ExternalInput"),
        nc.dram_tensor("WVs", [NW + 1, NW], f32, kind="ExternalInput"),
        nc.dram_tensor("WVt", [NW + 1, NW], f32, kind="ExternalInput"),
        nc.dram_tensor("ones", [1, N], f32, kind="ExternalInput"),
        nc.dram_tensor("Y", [N, H * D], f32, kind="ExternalOutput"),
        nc.dram_tensor("As_hat_t", [H, N, N], f32, kind="ExternalOutput"),
    )
    with tile.TileContext(nc) as tc:
        with ExitStack() as ctx:
            _emit(ctx, tc, nc, io)
    nc.compile()
    return nc


def _numpy_fallback(Dst, s, t, E, G, mask_s, Ws, bs, Wt, bt):
    b, n = s.shape[:2]
    qkvs = (s @ Ws + bs).reshape(b, n, 3 * D, H)
    qkvt = (t @ Wt + bt).reshape(b, n, 3 * D, H)
    Qs, Ks, Vs = qkvs[:, :, :D], qkvs[:, :, D : 2 * D], qkvs[:, :, 2 * D :]
    Qt, Kt, Vt = qkvt[:, :, :D], qkvt[:, :, D : 2 * D], qkvt[:, :, 2 * D :]
    E_T = np.transpose(E, (0, 2, 1, 3))
    G_T = np.transpose(G, (0, 2, 1, 3))
    As_hat = np.einsum("bldh,bmdh->blmh", Qs, Kt) * SCALE + E
    At_hat = np.einsum("bldh,bmdh->blmh", Qt, Ks) * SCALE + E_T
    As_t = As_hat * Dst[..., None]
    At_t = At_hat * np.transpose(Dst, (0, 2, 1))[..., None]
    p_s = 1.0 / (1.0 + np.exp(-(As_t - At_t)))
    p_t = 1.0 - p_s
    gates_s = 1.0 / (1.0 + np.exp(-(G + mask_s)))
    gates_t = 1.0 / (1.0 + np.exp(-(G_T + mask_s)))
    Y = np.einsum("blmh,bmkh->blkh", p_s * gates_s, Vt) + np.einsum(
        "blmh,bmkh->blkh", p_t * gates_t, Vs
    )
    return Y.reshape(b, n, H * D).astype(np.float32), As_hat.astype(np.float32)


def kernel(training, Dst, s, t, E, G, mask_s, Ws, bs, Wt, bt):
    global LAST_RESULTS
    Dst = np.asarray(Dst, np.float32)
    s = np.asarray(s, np.float32)
    t = np.asarray(t, np.float32)
    E = np.asarray(E, np.float32)
    G = np.asarray(G, np.float32)
    mask_s = np.asarray(mask_s, np.float32)

    if mask_s.any():
        # the device program hardcodes mask_s == 0 (true for this problem);
        # stay correct for any other caller
        return _numpy_fallback(
            Dst, s, t, E, G, mask_s,
            np.asarray(Ws, np.float32), np.asarray(bs, np.float32),
            np.asarray(Wt, np.float32), np.asarray(bt, np.float32),
        )

    from concourse.bass_utils import run_bass_kernel_spmd

    if "nc" not in _CACHE:
        _CACHE["nc"] = _build()
    nc = _CACHE["nc"]

    M1T, M2T, WVs, WVt = _prep_weights(Ws, bs, Wt, bt)

    in_maps = []
    for b in range(B):
        in_maps.append(
            {
                "E_t": np.ascontiguousarray(E[b].transpose(2, 0, 1)),
                "G_t": np.ascontiguousarray(G[b].transpose(2, 0, 1)),
                "Dst": np.ascontiguousarray(Dst[b]),
                "s": np.ascontiguousarray(s[b]),
                "t": np.ascontiguousarray(t[b]),
                "M1T": M1T,
                "M2T": M2T,
                "WVs": WVs,
                "WVt": WVt,
                "ones": np.ones((1, N), np.float32),
            }
        )

    res = run_bass_kernel_spmd(
        nc, in_maps, list(range(NCORES)),
        trace=bool(int(os.environ.get("KBENCH_TRACE", "0"))),
    )
    LAST_RESULTS = res
    Y = np.stack([r["Y"] for r in res.results])
    As_hat = np.stack(
        [np.ascontiguousarray(r["As_hat_t"].transpose(1, 2, 0)) for r in res.results]
    )
    return Y, As_hat


# revision 20
# speedup vs baseline: 1.4084x; 1.4084x over previous
"""Trainium2 Bass kernel for the Combine-Attention layer.

Math (per batch b, head h; N=512 nodes, D=6 dot dim):
    qkv_s = s @ Ws + bs ; qkv_t = t @ Wt + bt          (projections)
    As_hat[l,m] = (Qs.Kt)[l,m]*scale + E[l,m]
    At_hat[l,m] = (Qt.Ks)[l,m]*scale + E[m,l]
    As~ = As_hat * Dst ;  At~[l,m] = At_hat[l,m]*Dst[m,l]
    p_s = sigmoid(As~ - At~) ; p_t = 1 - p_s
    W1 = p_s * sigmoid(G) ; W2[l,m] = p_t[l,m] * sigmoid(G[m,l])
    Y = W1 @ Vt + W2 @ Vs
Outputs: (Y reshaped [n, H*D], As_hat [n,n,H]).

Device strategy: data-parallel over batch (1 batch per NeuronCore, 8 cores).
All score-space work is done in the transposed orientation [m, l]:
    U[l,m]    := At~[m,l] = ((Ks.Qt)[l,m]*scale + E[l,m]) * Dst[l,m]
    diffT[m,l] = A^T[m,l] - U[m,l]        (A := As~; A^T via PE transposes)
    W1T[m,l]  = sigmoid(diffT)*gs^T[m,l] ; W2T[m,l] = sigmoid(-diffT)*gs[m,l]
    Y^T[k,l]  = sum_m Vt[m,k]*W1T[m,l] + Vs[m,k]*W2T[m,l]
so every DRAM access is layout-native (host pre-permutes E,G to [H,N,N] and
the QKV weights into head-major column order with bias folded in as an
extra contraction row).
"""

import os
import numpy as np

B, N, NW, H, D = 8, 512, 48, 8, 6
P = 128
NT = N // P          # 4 strips of 128 rows
SCALE = D ** -0.5
NCORES = 8

_CACHE = {}
LAST_RESULTS = None


# ----------------------------------------------------------------- host prep
def _prep_weights(Ws, bs, Wt, bt):
    """Fold projections into per-head score matrices + V weights.

    Original column c = (x*6 + d)*8 + h for x in {Q=0,K=1,V=2}.
    M1_h = scale * Wq_s_aug @ Wk_t_aug^T  (49x49):  S1  = s' M1 t'^T
    M2_h = scale * Wk_s_aug @ Wq_t_aug^T  (49x49):  S2T = s' M2 t'^T
    Packed transposed per head-pair into [49, 4, 128] (head 2p at free cols
    0:49, head 2p+1 at 64:113) so psum rows land at legal matmul bases.
    WV_aug [49, 48] per side, V column j = h*6 + k.
    """
    def aug(W, b, cols):
        return np.vstack([W[:, cols], b[cols][None, :]]).astype(np.float64)

    Ws = np.asarray(Ws, np.float64)
    bs = np.asarray(bs, np.float64)
    Wt = np.asarray(Wt, np.float64)
    bt = np.asarray(bt, np.float64)
    M1T = np.zeros((NW + 1, NT, P), np.float32)
    M2T = np.zeros((NW + 1, NT, P), np.float32)
    for h in range(H):
        qcols = [(0 * D + d) * H + h for d in range(D)]
        kcols = [(1 * D + d) * H + h for d in range(D)]
        wq_s = aug(Ws, bs, qcols)
        wk_s = aug(Ws, bs, kcols)
        wq_t = aug(Wt, bt, qcols)
        wk_t = aug(Wt, bt, kcols)
        m1 = (SCALE * wq_s @ wk_t.T).T.astype(np.float32)
        m2 = (SCALE * wk_s @ wq_t.T).T.astype(np.float32)
        p, r = divmod(h, 2)
        M1T[:, p, r * 64 : r * 64 + NW + 1] = m1
        M2T[:, p, r * 64 : r * 64 + NW + 1] = m2
    v_cols = [(2 * D + k) * H + h for h in range(H) for k in range(D)]
    WVs = np.ascontiguousarray(aug(Ws, bs, v_cols), dtype=np.float32)
    WVt = np.ascontiguousarray(aug(Wt, bt, v_cols), dtype=np.float32)
    return M1T, M2T, WVs, WVt


# ------------------------------------------------------------- device kernel
def _emit(ctx, tc, nc, io):
    import concourse.bass as bass
    import concourse.mybir as mybir
    from concourse.masks import make_identity

    f32 = mybir.dt.float32
    f32r = mybir.dt.float32r
    AF = mybir.ActivationFunctionType
    OP = mybir.AluOpType

    def rc(ap):
        # reinterpret fp32 as float32r: single-pass PE matmul (4x fp32 rate)
        return ap.bitcast(f32r)

    E_t, G_t, Dst_d, s_d, t_d, M1T_d, M2T_d, WVs_d, WVt_d, ones_d, Y_d, AsH_d = io

    singles = ctx.enter_context(tc.tile_pool(name="singles", bufs=1))

    ident = singles.tile([P, P], f32)
    make_identity(nc, ident)
    ident_r = singles.tile([P, P], f32r)
    nc.vector.tensor_copy(ident_r, ident)

    # Dst strips resident for the whole batch: [128, strip, 512]
    Dst_sb = singles.tile([P, NT, N], f32)
    nc.sync.dma_start(out=Dst_sb, in_=Dst_d.ap().rearrange("(a p) m -> p a m", p=P))

    # --- projections -------------------------------------------------------
    xTs = {}    # side -> [49, 512] sbuf (transposed inputs, ones row appended)
    Vnat = {}   # side -> [128, strip, 48] sbuf (cols: h*6 + k)
    R = {}      # (1|2, pair) -> [128, 512] sbuf rows r*64..r*64+48 = head 2p+r
    with tc.tile_pool(name="proj_ps", bufs=2, space="PSUM") as proj_ps:
        for side, src, Wv_d in (("s", s_d, WVs_d), ("t", t_d, WVt_d)):
            wv = singles.tile([NW + 1, NW], f32r, tag=f"wv_{side}")
            nc.sync.dma_start(out=wv, in_=Wv_d.ap())

            x_nat = singles.tile([P, NT, NW], f32, tag=f"xnat_{side}")
            nc.sync.dma_start(
                out=x_nat, in_=src.ap().rearrange("(a p) c -> p a c", p=P)
            )
            # rows 0:49 = [x^T; ones]; for side s, duplicated at partition 64
            # so score matmuls match the base partition of odd-head R slabs
            nrows = 64 + NW + 1 if side == "s" else NW + 1
            xT = singles.tile([nrows, N], f32r, tag=f"xT_{side}")
            # engine memset at partition 48 is illegal (32-aligned bases
            # only) -- DMA the ones row in instead
            nc.sync.dma_start(out=xT[NW : NW + 1, :], in_=ones_d.ap())
            for a in range(NT):
                ps = proj_ps.tile([NW, P], f32, tag="tr")
                nc.tensor.transpose(ps, x_nat[:, a, :], ident)
                nc.vector.tensor_copy(xT[0:NW, a * P : (a + 1) * P], ps)
            if side == "s":
                nc.sync.dma_start(
                    out=xT[64 : 64 + NW + 1, :], in_=xT[0 : NW + 1, :]
                )
            xTs[side] = xT

            Vnat[side] = singles.tile(
                [P, NT, NW], f32r, tag=f"vnat_{side}", name=f"vnat_{side}"
            )
            for a in range(NT):
                vps = proj_ps.tile([P, NW], f32, tag="v")
                nc.tensor.matmul(
                    vps, lhsT=xT[0 : NW + 1, a * P : (a + 1) * P], rhs=wv,
                    start=True, stop=True,
                )
                nc.vector.tensor_copy(Vnat[side][:, a, :], vps)

        # R1/R2: per head-pair p, rows r*64..r*64+48 hold M @ t'^T for head 2p+r
        for idx, M_d in ((1, M1T_d), (2, M2T_d)):
            m_sb = singles.tile(
                [NW + 1, NT, P], f32r, tag=f"m{idx}", name=f"m{idx}_sb"
            )
            nc.sync.dma_start(out=m_sb, in_=M_d.ap())
            for p in range(NT):
                rps = proj_ps.tile([P, N], f32, tag="r")
                nc.tensor.matmul(
                    rps, lhsT=m_sb[:, p, :], rhs=xTs["t"],
                    start=True, stop=True,
                )
                r_sb = singles.tile(
                    [P, N], f32r, tag=f"r{idx}_{p}", name=f"r{idx}_{p}"
                )
                nc.vector.tensor_copy(r_sb, rps)
                R[(idx, p)] = r_sb

    Ystage = singles.tile([P, NT, H * D], f32)

    score_ps = ctx.enter_context(tc.tile_pool(name="score_ps", bufs=2, space="PSUM"))
    trans_ps = ctx.enter_context(tc.tile_pool(name="trans_ps", bufs=1, space="PSUM"))
    y_ps = ctx.enter_context(tc.tile_pool(name="y_ps", bufs=1, space="PSUM"))
    big = ctx.enter_context(tc.tile_pool(name="big", bufs=5))
    med = ctx.enter_context(tc.tile_pool(name="med", bufs=2))
    io_pool = ctx.enter_context(tc.tile_pool(name="io", bufs=3))

    for h in range(H):
        pair, r = divmod(h, 2)
        R1_h = R[(1, pair)][r * 64 : r * 64 + NW + 1, :]
        R2_h = R[(2, pair)][r * 64 : r * 64 + NW + 1, :]
        sT = xTs["s"][r * 64 : r * 64 + NW + 1, :]

        A_s, U_s, gs_s = [], [], []
        for i in range(NT):
            cols = slice(i * P, (i + 1) * P)
            E_i = io_pool.tile([P, N], f32, tag="E")
            nc.sync.dma_start(out=E_i, in_=E_t.ap()[h, i * P : (i + 1) * P, :])
            G_i = io_pool.tile([P, N], f32, tag="G")
            nc.sync.dma_start(out=G_i, in_=G_t.ap()[h, i * P : (i + 1) * P, :])

            ps1 = score_ps.tile([P, N], f32, tag="ps1")
            nc.tensor.matmul(ps1, lhsT=sT[:, cols], rhs=R1_h, start=True, stop=True)
            As_i = io_pool.tile([P, N], f32, tag="As")
            # As_hat = S1 + E  (fused psum-read + E add)
            nc.vector.scalar_tensor_tensor(
                out=As_i, in0=ps1, scalar=1.0, in1=E_i, op0=OP.mult, op1=OP.add
            )
            nc.sync.dma_start(out=AsH_d.ap()[h, i * P : (i + 1) * P, :], in_=As_i)

            A_i = big.tile([P, N], f32r, tag="A")
            nc.vector.tensor_mul(A_i, As_i, Dst_sb[:, i, :])
            A_s.append(A_i)

            ps2 = score_ps.tile([P, N], f32, tag="ps2")
            nc.tensor.matmul(ps2, lhsT=sT[:, cols], rhs=R2_h, start=True, stop=False)
            E_r = io_pool.tile([P, N], f32r, tag="Er")
            nc.sync.dma_start(
                out=E_r, in_=E_t.ap()[h, i * P : (i + 1) * P, :].bitcast(f32r)
            )
            nc.tensor.matmul(ps2, lhsT=ident_r, rhs=E_r, start=False, stop=True)
            U_i = big.tile([P, N], f32, tag="U")
            nc.vector.tensor_mul(U_i, ps2, Dst_sb[:, i, :])
            U_s.append(U_i)

            gs_i = big.tile([P, N], f32r, tag="gs")
            nc.scalar.activation(gs_i, G_i, AF.Sigmoid)
            gs_s.append(gs_i)

        W1T_s, W2T_s = [], []
        for j in range(NT):
            jcols = slice(j * P, (j + 1) * P)
            atp = trans_ps.tile([P, N], f32r, tag="at")
            gtp = trans_ps.tile([P, N], f32r, tag="gt")
            for i in range(NT):
                icols = slice(i * P, (i + 1) * P)
                nc.tensor.transpose(atp[:, icols], A_s[i][:, jcols], ident_r)
                nc.tensor.transpose(gtp[:, icols], gs_s[i][:, jcols], ident_r)
            diff = med.tile([P, N], f32, tag="diff")
            nc.vector.tensor_sub(diff, atp, U_s[j])
            p_j = med.tile([P, N], f32, tag="p")
            nc.scalar.activation(p_j, diff, AF.Sigmoid)
            q_j = med.tile([P, N], f32, tag="q")
            nc.scalar.activation(q_j, diff, AF.Sigmoid, scale=-1.0)
            W1T_j = big.tile([P, N], f32r, tag="W1")
            nc.vector.tensor_mul(W1T_j, p_j, gtp)
            W1T_s.append(W1T_j)
            W2T_j = big.tile([P, N], f32r, tag="W2")
            nc.vector.tensor_mul(W2T_j, q_j, gs_s[j])
            W2T_s.append(W2T_j)

        yt = y_ps.tile([D, N], f32, tag="yt")
        for j in range(NT):
            nc.tensor.matmul(
                yt, lhsT=Vnat["t"][:, j, h * D : (h + 1) * D], rhs=W1T_s[j],
                start=(j == 0), stop=False,
            )
            nc.tensor.matmul(
                yt, lhsT=Vnat["s"][:, j, h * D : (h + 1) * D], rhs=W2T_s[j],
                start=False, stop=(j == NT - 1),
            )
        yt_sb = med.tile([D, N], f32, tag="ytsb")
        nc.vector.tensor_copy(yt_sb, yt)
        yt2 = y_ps.tile([P, NT, D], f32, tag="yt2")
        for i in range(NT):
            nc.tensor.transpose(
                yt2[:, i, :], yt_sb[:, i * P : (i + 1) * P], ident[:D, :D]
            )
        # Ystage columns k*8+h  <-  yt2 columns k
        nc.vector.tensor_copy(Ystage[:, :, h :: H], yt2)

    nc.sync.dma_start(
        out=Y_d.ap().rearrange("(a p) c -> p a c", p=P), in_=Ystage
    )


def _build():
    from contextlib import ExitStack

    import concourse.bacc as bacc
    import concourse.mybir as mybir
    import concourse.tile as tile

    f32 = mybir.dt.float32
    f32r = mybir.dt.float32r
    nc = bacc.Bacc("TRN2", target_bir_lowering=False, debug=False)
    io = (
        nc.dram_tensor("E_t", [H, N, N], f32, kind="ExternalInput"),
        nc.dram_tensor("G_t", [H, N, N], f32, kind="ExternalInput"),
        nc.dram_tensor("Dst", [N, N], f32, kind="ExternalInput"),
        nc.dram_tensor("s", [N, NW], f32, kind="ExternalInput"),
        nc.dram_tensor("t", [N, NW], f32, kind="ExternalInput"),
        nc.dram_tensor("M1T", [NW + 1, NT, P], f32r, kind="ExternalInput"),
        nc.dram_tensor("M2T", [NW + 1, NT, P], f32r, kind="ExternalInput"),
        nc.dram_tensor("WVs", [NW + 1, NW], f32r, kind="ExternalInput"),
        nc.dram_tensor("WVt", [NW + 1, NW], f32r, kind="ExternalInput"),
        nc.dram_tensor("ones", [1, N], f32r, kind="ExternalInput"),
        nc.dram_tensor("Y", [N, H * D], f32, kind="ExternalOutput"),
        nc.dram_tensor("As_hat_t", [H, N, N], f32, kind="ExternalOutput"),
    )
    with tile.TileContext(nc) as tc:
        with ExitStack() as ctx:
            _emit(ctx, tc, nc, io)
    nc.compile()
    return nc


def _numpy_fallback(Dst, s, t, E, G, mask_s, Ws, bs, Wt, bt):
    b, n = s.shape[:2]
    qkvs = (s @ Ws + bs).reshape(b, n, 3 * D, H)
    qkvt = (t @ Wt + bt).reshape(b, n, 3 * D, H)
    Qs, Ks, Vs = qkvs[:, :, :D], qkvs[:, :, D : 2 * D], qkvs[:, :, 2 * D :]
    Qt, Kt, Vt = qkvt[:, :, :D], qkvt[:, :, D : 2 * D], qkvt[:, :, 2 * D :]
    E_T = np.transpose(E, (0, 2, 1, 3))
    G_T = np.transpose(G, (0, 2, 1, 3))
    As_hat = np.einsum("bldh,bmdh->blmh", Qs, Kt) * SCALE + E
    At_hat = np.einsum("bldh,bmdh->blmh", Qt, Ks) * SCALE + E_T
    As_t = As_hat * Dst[..., None]
    At_t = At_hat * np.transpose(Dst, (0, 2, 1))[..., None]
    p_s = 1.0 / (1.0 + np.exp(-(As_t - At_t)))
    p_t = 1.0 - p_s
    gates_s = 1.0 / (1.0 + np.exp(-(G + mask_s)))
    gates_t = 1.0 / (1.0 + np.exp(-(G_T + mask_s)))
    Y = np.einsum("blmh,bmkh->blkh", p_s * gates_s, Vt) + np.einsum(
        "blmh,bmkh->blkh", p_t * gates_t, Vs
    )
    return Y.reshape(b, n, H * D).astype(np.float32), As_hat.astype(np.float32)


def kernel(training, Dst, s, t, E, G, mask_s, Ws, bs, Wt, bt):
    global LAST_RESULTS
    Dst = np.asarray(Dst, np.float32)
    s = np.asarray(s, np.float32)
    t = np.asarray(t, np.float32)
    E = np.asarray(E, np.float32)
    G = np.asarray(G, np.float32)
    mask_s = np.asarray(mask_s, np.float32)

    if mask_s.any():
        # the device program hardcodes mask_s == 0 (true for this problem);
        # stay correct for any other caller
        return _numpy_fallback(
            Dst, s, t, E, G, mask_s,
            np.asarray(Ws, np.float32), np.asarray(bs, np.float32),
            np.asarray(Wt, np.float32), np.asarray(bt, np.float32),
        )

    from concourse.bass_utils import run_bass_kernel_spmd

    if "nc" not in _CACHE:
        _CACHE["nc"] = _build()
    nc = _CACHE["nc"]

    M1T, M2T, WVs, WVt = _prep_weights(Ws, bs, Wt, bt)

    in_maps = []
    for b in range(B):
        in_maps.append(
            {
                "E_t": np.ascontiguousarray(E[b].transpose(2, 0, 1)),
                "G_t": np.ascontiguousarray(G[b].transpose(2, 0, 1)),
                "Dst": np.ascontiguousarray(Dst[b]),
                "s": np.ascontiguousarray(s[b]),
                "t": np.ascontiguousarray(t[b]),
                "M1T": M1T,
                "M2T": M2T,
                "WVs": WVs,
                "WVt": WVt,
                "ones": np.ones((1, N), np.float32),
            }
        )

    res = run_bass_kernel_spmd(
        nc, in_maps, list(range(NCORES)),
        trace=bool(int(os.environ.get("KBENCH_TRACE", "0"))),
    )
    LAST_RESULTS = res
    Y = np.stack([r["Y"] for r in res.results])
    As_hat = np.stack(
        [np.ascontiguousarray(r["As_hat_t"].transpose(1, 2, 0)) for r in res.results]
    )
    return Y, As_hat


# revision 21
# speedup vs baseline: 1.4246x; 1.0115x over previous
"""Trainium2 Bass kernel for the Combine-Attention layer.

Math (per batch b, head h; N=512 nodes, D=6 dot dim):
    qkv_s = s @ Ws + bs ; qkv_t = t @ Wt + bt          (projections)
    As_hat[l,m] = (Qs.Kt)[l,m]*scale + E[l,m]
    At_hat[l,m] = (Qt.Ks)[l,m]*scale + E[m,l]
    As~ = As_hat * Dst ;  At~[l,m] = At_hat[l,m]*Dst[m,l]
    p_s = sigmoid(As~ - At~) ; p_t = 1 - p_s
    W1 = p_s * sigmoid(G) ; W2[l,m] = p_t[l,m] * sigmoid(G[m,l])
    Y = W1 @ Vt + W2 @ Vs
Outputs: (Y reshaped [n, H*D], As_hat [n,n,H]).

Device strategy: data-parallel over batch (1 batch per NeuronCore, 8 cores).
All score-space work is done in the transposed orientation [m, l]:
    U[l,m]    := At~[m,l] = ((Ks.Qt)[l,m]*scale + E[l,m]) * Dst[l,m]
    diffT[m,l] = A^T[m,l] - U[m,l]        (A := As~; A^T via PE transposes)
    W1T[m,l]  = sigmoid(diffT)*gs^T[m,l] ; W2T[m,l] = sigmoid(-diffT)*gs[m,l]
    Y^T[k,l]  = sum_m Vt[m,k]*W1T[m,l] + Vs[m,k]*W2T[m,l]
so every DRAM access is layout-native (host pre-permutes E,G to [H,N,N] and
the QKV weights into head-major column order with bias folded in as an
extra contraction row).
"""

import os
import numpy as np

B, N, NW, H, D = 8, 512, 48, 8, 6
P = 128
NT = N // P          # 4 strips of 128 rows
SCALE = D ** -0.5
NCORES = 8

_CACHE = {}
LAST_RESULTS = None


# ----------------------------------------------------------------- host prep
def _prep_weights(Ws, bs, Wt, bt):
    """Fold projections into per-head score matrices + V weights.

    Original column c = (x*6 + d)*8 + h for x in {Q=0,K=1,V=2}.
    M1_h = scale * Wq_s_aug @ Wk_t_aug^T  (49x49):  S1  = s' M1 t'^T
    M2_h = scale * Wk_s_aug @ Wq_t_aug^T  (49x49):  S2T = s' M2 t'^T
    Packed transposed per head-pair into [49, 4, 128] (head 2p at free cols
    0:49, head 2p+1 at 64:113) so psum rows land at legal matmul bases.
    WV_aug [49, 48] per side, V column j = h*6 + k.
    """
    def aug(W, b, cols):
        return np.vstack([W[:, cols], b[cols][None, :]]).astype(np.float64)

    Ws = np.asarray(Ws, np.float64)
    bs = np.asarray(bs, np.float64)
    Wt = np.asarray(Wt, np.float64)
    bt = np.asarray(bt, np.float64)
    M1T = np.zeros((NW + 1, NT, P), np.float32)
    M2T = np.zeros((NW + 1, NT, P), np.float32)
    for h in range(H):
        qcols = [(0 * D + d) * H + h for d in range(D)]
        kcols = [(1 * D + d) * H + h for d in range(D)]
        wq_s = aug(Ws, bs, qcols)
        wk_s = aug(Ws, bs, kcols)
        wq_t = aug(Wt, bt, qcols)
        wk_t = aug(Wt, bt, kcols)
        m1 = (SCALE * wq_s @ wk_t.T).T.astype(np.float32)
        m2 = (SCALE * wk_s @ wq_t.T).T.astype(np.float32)
        p, r = divmod(h, 2)
        M1T[:, p, r * 64 : r * 64 + NW + 1] = m1
        M2T[:, p, r * 64 : r * 64 + NW + 1] = m2
    v_cols = [(2 * D + k) * H + h for h in range(H) for k in range(D)]
    WVs = np.ascontiguousarray(aug(Ws, bs, v_cols), dtype=np.float32)
    WVt = np.ascontiguousarray(aug(Wt, bt, v_cols), dtype=np.float32)
    return M1T, M2T, WVs, WVt


# ------------------------------------------------------------- device kernel
def _emit(ctx, tc, nc, io):
    import concourse.bass as bass
    import concourse.mybir as mybir
    from concourse.masks import make_identity

    f32 = mybir.dt.float32
    f32r = mybir.dt.float32r
    AF = mybir.ActivationFunctionType
    OP = mybir.AluOpType

    def rc(ap):
        # reinterpret fp32 as float32r: single-pass PE matmul (4x fp32 rate)
        return ap.bitcast(f32r)

    E_t, G_t, Dst_d, s_d, t_d, M1T_d, M2T_d, WVs_d, WVt_d, ones_d, Y_d, AsH_d = io

    singles = ctx.enter_context(tc.tile_pool(name="singles", bufs=1))

    ident = singles.tile([P, P], f32)
    make_identity(nc, ident)
    ident_r = singles.tile([P, P], f32r)
    nc.vector.tensor_copy(ident_r, ident)

    # Dst strips resident for the whole batch: [128, strip, 512]
    Dst_sb = singles.tile([P, NT, N], f32)
    nc.sync.dma_start(out=Dst_sb, in_=Dst_d.ap().rearrange("(a p) m -> p a m", p=P))

    # --- projections -------------------------------------------------------
    xTs = {}    # side -> [49, 512] sbuf (transposed inputs, ones row appended)
    Vnat = {}   # side -> [128, strip, 48] sbuf (cols: h*6 + k)
    R = {}      # (1|2, pair) -> [128, 512] sbuf rows r*64..r*64+48 = head 2p+r
    with tc.tile_pool(name="proj_ps", bufs=2, space="PSUM") as proj_ps:
        for side, src, Wv_d in (("s", s_d, WVs_d), ("t", t_d, WVt_d)):
            wv = singles.tile([NW + 1, NW], f32r, tag=f"wv_{side}")
            nc.sync.dma_start(out=wv, in_=Wv_d.ap())

            x_nat = singles.tile([P, NT, NW], f32, tag=f"xnat_{side}")
            nc.sync.dma_start(
                out=x_nat, in_=src.ap().rearrange("(a p) c -> p a c", p=P)
            )
            # rows 0:49 = [x^T; ones]; for side s, duplicated at partition 64
            # so score matmuls match the base partition of odd-head R slabs
            nrows = 64 + NW + 1 if side == "s" else NW + 1
            xT = singles.tile([nrows, N], f32r, tag=f"xT_{side}")
            # engine memset at partition 48 is illegal (32-aligned bases
            # only) -- DMA the ones row in instead
            nc.sync.dma_start(out=xT[NW : NW + 1, :], in_=ones_d.ap())
            for a in range(NT):
                ps = proj_ps.tile([NW, P], f32, tag="tr")
                nc.tensor.transpose(ps, x_nat[:, a, :], ident)
                nc.vector.tensor_copy(xT[0:NW, a * P : (a + 1) * P], ps)
            if side == "s":
                nc.sync.dma_start(
                    out=xT[64 : 64 + NW + 1, :], in_=xT[0 : NW + 1, :]
                )
            xTs[side] = xT

            Vnat[side] = singles.tile(
                [P, NT, NW], f32r, tag=f"vnat_{side}", name=f"vnat_{side}"
            )
            for a in range(NT):
                vps = proj_ps.tile([P, NW], f32, tag="v")
                nc.tensor.matmul(
                    vps, lhsT=xT[0 : NW + 1, a * P : (a + 1) * P], rhs=wv,
                    start=True, stop=True,
                )
                nc.vector.tensor_copy(Vnat[side][:, a, :], vps)

        # R1/R2: per head-pair p, rows r*64..r*64+48 hold M @ t'^T for head 2p+r
        for idx, M_d in ((1, M1T_d), (2, M2T_d)):
            m_sb = singles.tile(
                [NW + 1, NT, P], f32r, tag=f"m{idx}", name=f"m{idx}_sb"
            )
            nc.sync.dma_start(out=m_sb, in_=M_d.ap())
            for p in range(NT):
                rps = proj_ps.tile([P, N], f32, tag="r")
                nc.tensor.matmul(
                    rps, lhsT=m_sb[:, p, :], rhs=xTs["t"],
                    start=True, stop=True,
                )
                r_sb = singles.tile(
                    [P, N], f32r, tag=f"r{idx}_{p}", name=f"r{idx}_{p}"
                )
                nc.vector.tensor_copy(r_sb, rps)
                R[(idx, p)] = r_sb

    Ystage = singles.tile([P, NT, H * D], f32)

    cfg = dict(
        kv.split(":")
        for kv in os.environ.get("KBENCH_BUFS", "").split(",")
        if ":" in kv
    )

    def nb(key, dflt):
        return int(cfg.get(key, dflt))

    score_ps = ctx.enter_context(
        tc.tile_pool(name="score_ps", bufs=nb("score", 2), space="PSUM")
    )
    trans_ps = ctx.enter_context(
        tc.tile_pool(name="trans_ps", bufs=nb("trans", 1), space="PSUM")
    )
    y_ps = ctx.enter_context(
        tc.tile_pool(name="y_ps", bufs=nb("y", 1), space="PSUM")
    )
    big = ctx.enter_context(tc.tile_pool(name="big", bufs=nb("big", 5)))
    med = ctx.enter_context(tc.tile_pool(name="med", bufs=nb("med", 2)))
    io_pool = ctx.enter_context(tc.tile_pool(name="io", bufs=nb("io", 3)))

    for h in range(H):
        pair, r = divmod(h, 2)
        R1_h = R[(1, pair)][r * 64 : r * 64 + NW + 1, :]
        R2_h = R[(2, pair)][r * 64 : r * 64 + NW + 1, :]
        sT = xTs["s"][r * 64 : r * 64 + NW + 1, :]

        A_s, U_s, gs_s = [], [], []
        for i in range(NT):
            cols = slice(i * P, (i + 1) * P)
            E_i = io_pool.tile([P, N], f32, tag="E")
            nc.sync.dma_start(out=E_i, in_=E_t.ap()[h, i * P : (i + 1) * P, :])
            G_i = io_pool.tile([P, N], f32, tag="G")
            nc.sync.dma_start(out=G_i, in_=G_t.ap()[h, i * P : (i + 1) * P, :])

            ps1 = score_ps.tile([P, N], f32, tag="ps1")
            nc.tensor.matmul(ps1, lhsT=sT[:, cols], rhs=R1_h, start=True, stop=True)
            As_i = io_pool.tile([P, N], f32, tag="As")
            # As_hat = S1 + E  (fused psum-read + E add)
            nc.vector.scalar_tensor_tensor(
                out=As_i, in0=ps1, scalar=1.0, in1=E_i, op0=OP.mult, op1=OP.add
            )
            nc.sync.dma_start(out=AsH_d.ap()[h, i * P : (i + 1) * P, :], in_=As_i)

            A_i = big.tile([P, N], f32r, tag="A")
            nc.vector.tensor_mul(A_i, As_i, Dst_sb[:, i, :])
            A_s.append(A_i)

            ps2 = score_ps.tile([P, N], f32, tag="ps2")
            nc.tensor.matmul(ps2, lhsT=sT[:, cols], rhs=R2_h, start=True, stop=False)
            E_r = io_pool.tile([P, N], f32r, tag="Er")
            nc.sync.dma_start(
                out=E_r, in_=E_t.ap()[h, i * P : (i + 1) * P, :].bitcast(f32r)
            )
            nc.tensor.matmul(ps2, lhsT=ident_r, rhs=E_r, start=False, stop=True)
            U_i = big.tile([P, N], f32, tag="U")
            nc.vector.tensor_mul(U_i, ps2, Dst_sb[:, i, :])
            U_s.append(U_i)

            gs_i = big.tile([P, N], f32r, tag="gs")
            nc.scalar.activation(gs_i, G_i, AF.Sigmoid)
            gs_s.append(gs_i)

        W1T_s, W2T_s = [], []
        for j in range(NT):
            jcols = slice(j * P, (j + 1) * P)
            atp = trans_ps.tile([P, N], f32r, tag="at")
            gtp = trans_ps.tile([P, N], f32r, tag="gt")
            for i in range(NT):
                icols = slice(i * P, (i + 1) * P)
                nc.tensor.transpose(atp[:, icols], A_s[i][:, jcols], ident_r)
                nc.tensor.transpose(gtp[:, icols], gs_s[i][:, jcols], ident_r)
            diff = med.tile([P, N], f32, tag="diff")
            nc.vector.tensor_sub(diff, atp, U_s[j])
            p_j = med.tile([P, N], f32, tag="p")
            nc.scalar.activation(p_j, diff, AF.Sigmoid)
            q_j = med.tile([P, N], f32, tag="q")
            nc.scalar.activation(q_j, diff, AF.Sigmoid, scale=-1.0)
            W1T_j = big.tile([P, N], f32r, tag="W1")
            nc.vector.tensor_mul(W1T_j, p_j, gtp)
            W1T_s.append(W1T_j)
            W2T_j = big.tile([P, N], f32r, tag="W2")
            nc.vector.tensor_mul(W2T_j, q_j, gs_s[j])
            W2T_s.append(W2T_j)

        yt = y_ps.tile([D, N], f32, tag="yt")
        for j in range(NT):
            nc.tensor.matmul(
                yt, lhsT=Vnat["t"][:, j, h * D : (h + 1) * D], rhs=W1T_s[j],
                start=(j == 0), stop=False,
            )
            nc.tensor.matmul(
                yt, lhsT=Vnat["s"][:, j, h * D : (h + 1) * D], rhs=W2T_s[j],
                start=False, stop=(j == NT - 1),
            )
        yt_sb = med.tile([D, N], f32, tag="ytsb")
        nc.vector.tensor_copy(yt_sb, yt)
        yt2 = y_ps.tile([P, NT, D], f32, tag="yt2")
        for i in range(NT):
            nc.tensor.transpose(
                yt2[:, i, :], yt_sb[:, i * P : (i + 1) * P], ident[:D, :D]
            )
        # Ystage columns k*8+h  <-  yt2 columns k
        nc.vector.tensor_copy(Ystage[:, :, h :: H], yt2)

    nc.sync.dma_start(
        out=Y_d.ap().rearrange("(a p) c -> p a c", p=P), in_=Ystage
    )


def _build():
    from contextlib import ExitStack

    import concourse.bacc as bacc
    import concourse.mybir as mybir
    import concourse.tile as tile

    f32 = mybir.dt.float32
    f32r = mybir.dt.float32r
    nc = bacc.Bacc("TRN2", target_bir_lowering=False, debug=False)
    io = (
        nc.dram_tensor("E_t", [H, N, N], f32, kind="ExternalInput"),
        nc.dram_tensor("G_t", [H, N, N], f32, kind="ExternalInput"),
        nc.dram_tensor("Dst", [N, N], f32, kind="ExternalInput"),
        nc.dram_tensor("s", [N, NW], f32, kind="ExternalInput"),
        nc.dram_tensor("t", [N, NW], f32, kind="ExternalInput"),
        nc.dram_tensor("M1T", [NW + 1, NT, P], f32r, kind="ExternalInput"),
        nc.dram_tensor("M2T", [NW + 1, NT, P], f32r, kind="ExternalInput"),
        nc.dram_tensor("WVs", [NW + 1, NW], f32r, kind="ExternalInput"),
        nc.dram_tensor("WVt", [NW + 1, NW], f32r, kind="ExternalInput"),
        nc.dram_tensor("ones", [1, N], f32r, kind="ExternalInput"),
        nc.dram_tensor("Y", [N, H * D], f32, kind="ExternalOutput"),
        nc.dram_tensor("As_hat_t", [H, N, N], f32, kind="ExternalOutput"),
    )
    with tile.TileContext(nc) as tc:
        with ExitStack() as ctx:
            _emit(ctx, tc, nc, io)
    nc.compile()
    return nc


def _numpy_fallback(Dst, s, t, E, G, mask_s, Ws, bs, Wt, bt):
    b, n = s.shape[:2]
    qkvs = (s @ Ws + bs).reshape(b, n, 3 * D, H)
    qkvt = (t @ Wt + bt).reshape(b, n, 3 * D, H)
    Qs, Ks, Vs = qkvs[:, :, :D], qkvs[:, :, D : 2 * D], qkvs[:, :, 2 * D :]
    Qt, Kt, Vt = qkvt[:, :, :D], qkvt[:, :, D : 2 * D], qkvt[:, :, 2 * D :]
    E_T = np.transpose(E, (0, 2, 1, 3))
    G_T = np.transpose(G, (0, 2, 1, 3))
    As_hat = np.einsum("bldh,bmdh->blmh", Qs, Kt) * SCALE + E
    At_hat = np.einsum("bldh,bmdh->blmh", Qt, Ks) * SCALE + E_T
    As_t = As_hat * Dst[..., None]
    At_t = At_hat * np.transpose(Dst, (0, 2, 1))[..., None]
    p_s = 1.0 / (1.0 + np.exp(-(As_t - At_t)))
    p_t = 1.0 - p_s
    gates_s = 1.0 / (1.0 + np.exp(-(G + mask_s)))
    gates_t = 1.0 / (1.0 + np.exp(-(G_T + mask_s)))
    Y = np.einsum("blmh,bmkh->blkh", p_s * gates_s, Vt) + np.einsum(
        "blmh,bmkh->blkh", p_t * gates_t, Vs
    )
    return Y.reshape(b, n, H * D).astype(np.float32), As_hat.astype(np.float32)


def kernel(training, Dst, s, t, E, G, mask_s, Ws, bs, Wt, bt):
    global LAST_RESULTS
    Dst = np.asarray(Dst, np.float32)
    s = np.asarray(s, np.float32)
    t = np.asarray(t, np.float32)
    E = np.asarray(E, np.float32)
    G = np.asarray(G, np.float32)
    mask_s = np.asarray(mask_s, np.float32)

    if mask_s.any():
        # the device program hardcodes mask_s == 0 (true for this problem);
        # stay correct for any other caller
        return _numpy_fallback(
            Dst, s, t, E, G, mask_s,
            np.asarray(Ws, np.float32), np.asarray(bs, np.float32),
            np.asarray(Wt, np.float32), np.asarray(bt, np.float32),
        )

    from concourse.bass_utils import run_bass_kernel_spmd

    if "nc" not in _CACHE:
        _CACHE["nc"] = _build()
    nc = _CACHE["nc"]

    M1T, M2T, WVs, WVt = _prep_weights(Ws, bs, Wt, bt)

    in_maps = []
    for b in range(B):
        in_maps.append(
            {
                "E_t": np.ascontiguousarray(E[b].transpose(2, 0, 1)),
                "G_t": np.ascontiguousarray(G[b].transpose(2, 0, 1)),
                "Dst": np.ascontiguousarray(Dst[b]),
                "s": np.ascontiguousarray(s[b]),
                "t": np.ascontiguousarray(t[b]),
                "M1T": M1T,
                "M2T": M2T,
                "WVs": WVs,
                "WVt": WVt,
                "ones": np.ones((1, N), np.float32),
            }
        )

    res = run_bass_kernel_spmd(
        nc, in_maps, list(range(NCORES)),
        trace=bool(int(os.environ.get("KBENCH_TRACE", "0"))),
    )
    LAST_RESULTS = res
    Y = np.stack([r["Y"] for r in res.results])
    As_hat = np.stack(
        [np.ascontiguousarray(r["As_hat_t"].transpose(1, 2, 0)) for r in res.results]
    )
    return Y, As_hat


# revision 23
# speedup vs baseline: 1.4874x; 1.0441x over previous
"""Trainium2 Bass kernel for the Combine-Attention layer.

Math (per batch b, head h; N=512 nodes, D=6 dot dim):
    qkv_s = s @ Ws + bs ; qkv_t = t @ Wt + bt          (projections)
    As_hat[l,m] = (Qs.Kt)[l,m]*scale + E[l,m]
    At_hat[l,m] = (Qt.Ks)[l,m]*scale + E[m,l]
    As~ = As_hat * Dst ;  At~[l,m] = At_hat[l,m]*Dst[m,l]
    p_s = sigmoid(As~ - At~) ; p_t = 1 - p_s
    W1 = p_s * sigmoid(G) ; W2[l,m] = p_t[l,m] * sigmoid(G[m,l])
    Y = W1 @ Vt + W2 @ Vs
Outputs: (Y reshaped [n, H*D], As_hat [n,n,H]).

Device strategy: data-parallel over batch (1 batch per NeuronCore, 8 cores).
All score-space work is done in the transposed orientation [m, l]:
    U[l,m]    := At~[m,l] = ((Ks.Qt)[l,m]*scale + E[l,m]) * Dst[l,m]
    diffT[m,l] = A^T[m,l] - U[m,l]        (A := As~; A^T via PE transposes)
    W1T[m,l]  = sigmoid(diffT)*gs^T[m,l] ; W2T[m,l] = sigmoid(-diffT)*gs[m,l]
    Y^T[k,l]  = sum_m Vt[m,k]*W1T[m,l] + Vs[m,k]*W2T[m,l]
so every DRAM access is layout-native (host pre-permutes E,G to [H,N,N] and
the QKV weights into head-major column order with bias folded in as an
extra contraction row).
"""

import os
import numpy as np

B, N, NW, H, D = 8, 512, 48, 8, 6
P = 128
NT = N // P          # 4 strips of 128 rows
SCALE = D ** -0.5
NCORES = 8

_CACHE = {}
LAST_RESULTS = None


# ----------------------------------------------------------------- host prep
def _prep_weights(Ws, bs, Wt, bt):
    """Fold projections into per-head score matrices + V weights.

    Original column c = (x*6 + d)*8 + h for x in {Q=0,K=1,V=2}.
    M1_h = scale * Wq_s_aug @ Wk_t_aug^T  (49x49):  S1  = s' M1 t'^T
    M2_h = scale * Wk_s_aug @ Wq_t_aug^T  (49x49):  S2T = s' M2 t'^T
    Packed transposed per head-pair into [49, 4, 128] (head 2p at free cols
    0:49, head 2p+1 at 64:113) so psum rows land at legal matmul bases.
    WV_aug [49, 48] per side, V column j = h*6 + k.
    """
    def aug(W, b, cols):
        return np.vstack([W[:, cols], b[cols][None, :]]).astype(np.float64)

    Ws = np.asarray(Ws, np.float64)
    bs = np.asarray(bs, np.float64)
    Wt = np.asarray(Wt, np.float64)
    bt = np.asarray(bt, np.float64)
    M1T = np.zeros((NW + 1, NT, P), np.float32)
    M2T = np.zeros((NW + 1, NT, P), np.float32)
    for h in range(H):
        qcols = [(0 * D + d) * H + h for d in range(D)]
        kcols = [(1 * D + d) * H + h for d in range(D)]
        wq_s = aug(Ws, bs, qcols)
        wk_s = aug(Ws, bs, kcols)
        wq_t = aug(Wt, bt, qcols)
        wk_t = aug(Wt, bt, kcols)
        m1 = (SCALE * wq_s @ wk_t.T).T.astype(np.float32)
        m2 = (SCALE * wk_s @ wq_t.T).T.astype(np.float32)
        p, r = divmod(h, 2)
        M1T[:, p, r * 64 : r * 64 + NW + 1] = m1
        M2T[:, p, r * 64 : r * 64 + NW + 1] = m2
    v_cols = [(2 * D + k) * H + h for h in range(H) for k in range(D)]
    WVs = np.ascontiguousarray(aug(Ws, bs, v_cols), dtype=np.float32)
    WVt = np.ascontiguousarray(aug(Wt, bt, v_cols), dtype=np.float32)
    return M1T, M2T, WVs, WVt


# ------------------------------------------------------------- device kernel
def _emit(ctx, tc, nc, io):
    import concourse.bass as bass
    import concourse.mybir as mybir
    from concourse.masks import make_identity

    f32 = mybir.dt.float32
    f32r = mybir.dt.float32r
    bf16 = mybir.dt.bfloat16
    AF = mybir.ActivationFunctionType
    OP = mybir.AluOpType
    use_bf = os.environ.get("KBENCH_BF16", "1") == "1"
    # combine-pipeline dtype: bf16 halves DVE cost (2x mode) and replaces
    # PE tile-transposes with DMA xbar transposes
    cdt = bf16 if use_bf else f32r

    def rc(ap):
        # reinterpret fp32 as float32r: single-pass PE matmul (4x fp32 rate)
        return ap.bitcast(f32r)

    E_t, G_t, Dst_d, s_d, t_d, M1T_d, M2T_d, WVs_d, WVt_d, ones_d, Y_d, AsH_d = io

    singles = ctx.enter_context(tc.tile_pool(name="singles", bufs=1))

    ident = singles.tile([P, P], f32)
    make_identity(nc, ident)
    ident_r = singles.tile([P, P], f32r)
    nc.vector.tensor_copy(ident_r, ident)
    ident_b = singles.tile([P, P], bf16)
    nc.vector.tensor_copy(ident_b, ident)

    # Dst strips resident for the whole batch: [128, strip, 512]
    Dst_sb = singles.tile([P, NT, N], f32)
    nc.sync.dma_start(out=Dst_sb, in_=Dst_d.ap().rearrange("(a p) m -> p a m", p=P))

    # --- projections -------------------------------------------------------
    xTs = {}    # side -> [49, 512] sbuf (transposed inputs, ones row appended)
    Vnat = {}   # side -> [128, strip, 48] sbuf (cols: h*6 + k)
    R = {}      # (1|2, pair) -> [128, 512] sbuf rows r*64..r*64+48 = head 2p+r
    with tc.tile_pool(name="proj_ps", bufs=2, space="PSUM") as proj_ps:
        for side, src, Wv_d in (("s", s_d, WVs_d), ("t", t_d, WVt_d)):
            wv = singles.tile([NW + 1, NW], f32r, tag=f"wv_{side}")
            nc.sync.dma_start(out=wv, in_=Wv_d.ap())

            x_nat = singles.tile([P, NT, NW], f32, tag=f"xnat_{side}")
            nc.sync.dma_start(
                out=x_nat, in_=src.ap().rearrange("(a p) c -> p a c", p=P)
            )
            # rows 0:49 = [x^T; ones]; for side s, duplicated at partition 64
            # so score matmuls match the base partition of odd-head R slabs
            nrows = 64 + NW + 1 if side == "s" else NW + 1
            xT = singles.tile([nrows, N], f32r, tag=f"xT_{side}")
            # engine memset at partition 48 is illegal (32-aligned bases
            # only) -- DMA the ones row in instead
            nc.sync.dma_start(out=xT[NW : NW + 1, :], in_=ones_d.ap())
            for a in range(NT):
                ps = proj_ps.tile([NW, P], f32, tag="tr")
                nc.tensor.transpose(ps, x_nat[:, a, :], ident)
                nc.vector.tensor_copy(xT[0:NW, a * P : (a + 1) * P], ps)
            if side == "s":
                nc.sync.dma_start(
                    out=xT[64 : 64 + NW + 1, :], in_=xT[0 : NW + 1, :]
                )
            xTs[side] = xT

            Vnat[side] = singles.tile(
                [P, NT, NW], cdt, tag=f"vnat_{side}", name=f"vnat_{side}"
            )
            for a in range(NT):
                vps = proj_ps.tile([P, NW], f32, tag="v")
                nc.tensor.matmul(
                    vps, lhsT=xT[0 : NW + 1, a * P : (a + 1) * P], rhs=wv,
                    start=True, stop=True,
                )
                nc.vector.tensor_copy(Vnat[side][:, a, :], vps)

        # R1/R2: per head-pair p, rows r*64..r*64+48 hold M @ t'^T for head 2p+r
        for idx, M_d in ((1, M1T_d), (2, M2T_d)):
            m_sb = singles.tile(
                [NW + 1, NT, P], f32r, tag=f"m{idx}", name=f"m{idx}_sb"
            )
            nc.sync.dma_start(out=m_sb, in_=M_d.ap())
            for p in range(NT):
                rps = proj_ps.tile([P, N], f32, tag="r")
                nc.tensor.matmul(
                    rps, lhsT=m_sb[:, p, :], rhs=xTs["t"],
                    start=True, stop=True,
                )
                r_sb = singles.tile(
                    [P, N], f32r, tag=f"r{idx}_{p}", name=f"r{idx}_{p}"
                )
                nc.vector.tensor_copy(r_sb, rps)
                R[(idx, p)] = r_sb

    Ystage = singles.tile([P, NT, H * D], f32)

    cfg = dict(
        kv.split(":")
        for kv in os.environ.get("KBENCH_BUFS", "").split(",")
        if ":" in kv
    )

    def nb(key, dflt):
        return int(cfg.get(key, dflt))

    score_ps = ctx.enter_context(
        tc.tile_pool(name="score_ps", bufs=nb("score", 2), space="PSUM")
    )
    trans_ps = ctx.enter_context(
        tc.tile_pool(name="trans_ps", bufs=nb("trans", 1), space="PSUM")
    )
    y_ps = ctx.enter_context(
        tc.tile_pool(name="y_ps", bufs=nb("y", 1), space="PSUM")
    )
    big = ctx.enter_context(tc.tile_pool(name="big", bufs=nb("big", 5)))
    med = ctx.enter_context(tc.tile_pool(name="med", bufs=nb("med", 2)))
    io_pool = ctx.enter_context(tc.tile_pool(name="io", bufs=nb("io", 3)))

    for h in range(H):
        pair, r = divmod(h, 2)
        R1_h = R[(1, pair)][r * 64 : r * 64 + NW + 1, :]
        R2_h = R[(2, pair)][r * 64 : r * 64 + NW + 1, :]
        sT = xTs["s"][r * 64 : r * 64 + NW + 1, :]

        A_s, U_s, gs_s = [], [], []
        for i in range(NT):
            cols = slice(i * P, (i + 1) * P)
            E_i = io_pool.tile([P, N], f32, tag="E")
            nc.sync.dma_start(out=E_i, in_=E_t.ap()[h, i * P : (i + 1) * P, :])
            G_i = io_pool.tile([P, N], f32, tag="G")
            nc.sync.dma_start(out=G_i, in_=G_t.ap()[h, i * P : (i + 1) * P, :])

            ps1 = score_ps.tile([P, N], f32, tag="ps1")
            nc.tensor.matmul(ps1, lhsT=sT[:, cols], rhs=R1_h, start=True, stop=True)
            As_i = io_pool.tile([P, N], f32, tag="As")
            # As_hat = S1 + E  (fused psum-read + E add)
            nc.vector.scalar_tensor_tensor(
                out=As_i, in0=ps1, scalar=1.0, in1=E_i, op0=OP.mult, op1=OP.add
            )
            nc.sync.dma_start(out=AsH_d.ap()[h, i * P : (i + 1) * P, :], in_=As_i)

            A_i = big.tile([P, N], cdt, tag="A")
            nc.vector.tensor_mul(A_i, As_i, Dst_sb[:, i, :])
            A_s.append(A_i)

            ps2 = score_ps.tile([P, N], f32, tag="ps2")
            nc.tensor.matmul(ps2, lhsT=sT[:, cols], rhs=R2_h, start=True, stop=False)
            E_r = io_pool.tile([P, N], f32r, tag="Er")
            nc.sync.dma_start(
                out=E_r, in_=E_t.ap()[h, i * P : (i + 1) * P, :].bitcast(f32r)
            )
            nc.tensor.matmul(ps2, lhsT=ident_r, rhs=E_r, start=False, stop=True)
            U_i = big.tile([P, N], cdt, tag="U")
            nc.vector.tensor_mul(U_i, ps2, Dst_sb[:, i, :])
            U_s.append(U_i)

            gs_i = big.tile([P, N], cdt, tag="gs")
            nc.scalar.activation(gs_i, G_i, AF.Sigmoid)
            gs_s.append(gs_i)

        W1T_s, W2T_s = [], []
        for j in range(NT):
            jcols = slice(j * P, (j + 1) * P)
            atp = trans_ps.tile([P, N], cdt, tag="at")
            gtp = trans_ps.tile([P, N], cdt, tag="gt")
            tr_ident = ident_b if use_bf else ident_r
            for i in range(NT):
                icols = slice(i * P, (i + 1) * P)
                nc.tensor.transpose(atp[:, icols], A_s[i][:, jcols], tr_ident)
                nc.tensor.transpose(gtp[:, icols], gs_s[i][:, jcols], tr_ident)
            diff = med.tile([P, N], cdt, tag="diff")
            nc.vector.tensor_sub(diff, atp, U_s[j])
            p_j = med.tile([P, N], cdt, tag="p")
            nc.scalar.activation(p_j, diff, AF.Sigmoid)
            q_j = med.tile([P, N], cdt, tag="q")
            nc.scalar.activation(q_j, diff, AF.Sigmoid, scale=-1.0)
            W1T_j = big.tile([P, N], cdt, tag="W1")
            nc.vector.tensor_mul(W1T_j, p_j, gtp)
            W1T_s.append(W1T_j)
            W2T_j = big.tile([P, N], cdt, tag="W2")
            nc.vector.tensor_mul(W2T_j, q_j, gs_s[j])
            W2T_s.append(W2T_j)

        yt = y_ps.tile([D, N], f32, tag="yt")
        for j in range(NT):
            nc.tensor.matmul(
                yt, lhsT=Vnat["t"][:, j, h * D : (h + 1) * D], rhs=W1T_s[j],
                start=(j == 0), stop=False,
            )
            nc.tensor.matmul(
                yt, lhsT=Vnat["s"][:, j, h * D : (h + 1) * D], rhs=W2T_s[j],
                start=False, stop=(j == NT - 1),
            )
        yt_sb = med.tile([D, N], f32, tag="ytsb")
        nc.vector.tensor_copy(yt_sb, yt)
        yt2 = y_ps.tile([P, NT, D], f32, tag="yt2")
        for i in range(NT):
            nc.tensor.transpose(
                yt2[:, i, :], yt_sb[:, i * P : (i + 1) * P], ident[:D, :D]
            )
        # Ystage columns k*8+h  <-  yt2 columns k
        nc.vector.tensor_copy(Ystage[:, :, h :: H], yt2)

    nc.sync.dma_start(
        out=Y_d.ap().rearrange("(a p) c -> p a c", p=P), in_=Ystage
    )


def _build():
    from contextlib import ExitStack

    import concourse.bacc as bacc
    import concourse.mybir as mybir
    import concourse.tile as tile

    f32 = mybir.dt.float32
    f32r = mybir.dt.float32r
    nc = bacc.Bacc("TRN2", target_bir_lowering=False, debug=False)
    io = (
        nc.dram_tensor("E_t", [H, N, N], f32, kind="ExternalInput"),
        nc.dram_tensor("G_t", [H, N, N], f32, kind="ExternalInput"),
        nc.dram_tensor("Dst", [N, N], f32, kind="ExternalInput"),
        nc.dram_tensor("s", [N, NW], f32, kind="ExternalInput"),
        nc.dram_tensor("t", [N, NW], f32, kind="ExternalInput"),
        nc.dram_tensor("M1T", [NW + 1, NT, P], f32r, kind="ExternalInput"),
        nc.dram_tensor("M2T", [NW + 1, NT, P], f32r, kind="ExternalInput"),
        nc.dram_tensor("WVs", [NW + 1, NW], f32r, kind="ExternalInput"),
        nc.dram_tensor("WVt", [NW + 1, NW], f32r, kind="ExternalInput"),
        nc.dram_tensor("ones", [1, N], f32r, kind="ExternalInput"),
        nc.dram_tensor("Y", [N, H * D], f32, kind="ExternalOutput"),
        nc.dram_tensor("As_hat_t", [H, N, N], f32, kind="ExternalOutput"),
    )
    with tile.TileContext(nc) as tc:
        with ExitStack() as ctx:
            _emit(ctx, tc, nc, io)
    nc.compile()
    return nc


def _numpy_fallback(Dst, s, t, E, G, mask_s, Ws, bs, Wt, bt):
    b, n = s.shape[:2]
    qkvs = (s @ Ws + bs).reshape(b, n, 3 * D, H)
    qkvt = (t @ Wt + bt).reshape(b, n, 3 * D, H)
    Qs, Ks, Vs = qkvs[:, :, :D], qkvs[:, :, D : 2 * D], qkvs[:, :, 2 * D :]
    Qt, Kt, Vt = qkvt[:, :, :D], qkvt[:, :, D : 2 * D], qkvt[:, :, 2 * D :]
    E_T = np.transpose(E, (0, 2, 1, 3))
    G_T = np.transpose(G, (0, 2, 1, 3))
    As_hat = np.einsum("bldh,bmdh->blmh", Qs, Kt) * SCALE + E
    At_hat = np.einsum("bldh,bmdh->blmh", Qt, Ks) * SCALE + E_T
    As_t = As_hat * Dst[..., None]
    At_t = At_hat * np.transpose(Dst, (0, 2, 1))[..., None]
    p_s = 1.0 / (1.0 + np.exp(-(As_t - At_t)))
    p_t = 1.0 - p_s
    gates_s = 1.0 / (1.0 + np.exp(-(G + mask_s)))
    gates_t = 1.0 / (1.0 + np.exp(-(G_T + mask_s)))
    Y = np.einsum("blmh,bmkh->blkh", p_s * gates_s, Vt) + np.einsum(
        "blmh,bmkh->blkh", p_t * gates_t, Vs
    )
    return Y.reshape(b, n, H * D).astype(np.float32), As_hat.astype(np.float32)


def kernel(training, Dst, s, t, E, G, mask_s, Ws, bs, Wt, bt):
    global LAST_RESULTS
    Dst = np.asarray(Dst, np.float32)
    s = np.asarray(s, np.float32)
    t = np.asarray(t, np.float32)
    E = np.asarray(E, np.float32)
    G = np.asarray(G, np.float32)
    mask_s = np.asarray(mask_s, np.float32)

    if mask_s.any():
        # the device program hardcodes mask_s == 0 (true for this problem);
        # stay correct for any other caller
        return _numpy_fallback(
            Dst, s, t, E, G, mask_s,
            np.asarray(Ws, np.float32), np.asarray(bs, np.float32),
            np.asarray(Wt, np.float32), np.asarray(bt, np.float32),
        )

    from concourse.bass_utils import run_bass_kernel_spmd

    if "nc" not in _CACHE:
        _CACHE["nc"] = _build()
    nc = _CACHE["nc"]

    M1T, M2T, WVs, WVt = _prep_weights(Ws, bs, Wt, bt)

    in_maps = []
    for b in range(B):
        in_maps.append(
            {
                "E_t": np.ascontiguousarray(E[b].transpose(2, 0, 1)),
                "G_t": np.ascontiguousarray(G[b].transpose(2, 0, 1)),
                "Dst": np.ascontiguousarray(Dst[b]),
                "s": np.ascontiguousarray(s[b]),
                "t": np.ascontiguousarray(t[b]),
                "M1T": M1T,
                "M2T": M2T,
                "WVs": WVs,
                "WVt": WVt,
                "ones": np.ones((1, N), np.float32),
            }
        )

    res = run_bass_kernel_spmd(
        nc, in_maps, list(range(NCORES)),
        trace=bool(int(os.environ.get("KBENCH_TRACE", "0"))),
    )
    LAST_RESULTS = res
    Y = np.stack([r["Y"] for r in res.results])
    As_hat = np.stack(
        [np.ascontiguousarray(r["As_hat_t"].transpose(1, 2, 0)) for r in res.results]
    )
    return Y, As_hat
